# revision 10
# baseline (speedup 1.0000x reference)
"""Trainium2 Bass kernel for nn_AdaptiveTopKSelector (causal top-k masking).

kernel(index_scores [4,4096,4096] f32, top_k=512) ->
    (top_k_mask [4,4096,4096] bool, top_k_indices [4,4096,512] int32,
     sparsity f32 scalar)

Strategy (8 NeuronCores, sequence-parallel over seq_len_q):
  Each core takes all 4 batches x a contiguous 512-wide q-chunk
  (2048 rows of 4096 scores). Per 128-row tile:
    1. candidate predicate P = (j <= q) & (x > T[q]) where T[q] is a
       host-side statistical prior (function of row position only) chosen
       so that K <= #candidates <= CAP with ~6 sigma margin;
    2. prefix-sum positions + gpsimd local_scatter compact the candidate
       values (f32 moved as u16 pairs) and their column indices into
       CAP-sized buffers;
    3. K/8 rounds of max8 / max_index / match_replace extract the top-K
       values in exact descending order (ties resolved to the lower
       index by the ascending-scan semantics of max_index/match_replace,
       matching jax.lax.top_k);
    4. two more local_scatters invert the rank->buffer-slot permutation
       into top_k_indices; short rows (q+1 <= K) get their deterministic
       [q+1..K) tail filled from an iota;
    5. the boolean mask is scattered from the final indices in 1024-wide
       chunks.
  A per-row candidate count is exported; any row whose count falls
  outside [K, CAP] (prior violated - never observed for randn inputs) is
  recomputed exactly on the host.
"""

import numpy as np

B, SQ, SK, K, CAP = 4, 4096, 4096, 512, 672
NCORES = 8
QCHUNK = SQ // NCORES
R = B * QCHUNK

SENTINEL = -4.0e9
NEGBIG = -3.0e9

_CACHE = {}


# ----------------------------------------------------------------------
# host-side threshold prior
# ----------------------------------------------------------------------
def _norm_ppf(p):
    """Acklam's inverse normal CDF approximation (|rel err| < 1.2e-9)."""
    p = np.asarray(p, dtype=np.float64)
    a = [-3.969683028665376e+01, 2.209460984245205e+02, -2.759285104469687e+02,
         1.383577518672690e+02, -3.066479806614716e+01, 2.506628277459239e+00]
    b = [-5.447609879822406e+01, 1.615858368580409e+02, -1.556989798598866e+02,
         6.680131188771972e+01, -1.328068155288572e+01]
    c = [-7.784894002430293e-03, -3.223964580411365e-01, -2.400758277161838e+00,
         -2.549732539343734e+00, 4.374664141464968e+00, 2.938163982698783e+00]
    d = [7.784695709041462e-03, 3.224671290700398e-01, 2.445134137142996e+00,
         3.754408661907416e+00]
    out = np.empty_like(p)
    plow, phigh = 0.02425, 1 - 0.02425
    lo = p < plow
    hi = p > phigh
    mid = ~(lo | hi)
    if lo.any():
        q = np.sqrt(-2 * np.log(p[lo]))
        out[lo] = ((((((c[0] * q + c[1]) * q + c[2]) * q + c[3]) * q + c[4]) * q + c[5])
                   / ((((d[0] * q + d[1]) * q + d[2]) * q + d[3]) * q + 1))
    if hi.any():
        q = np.sqrt(-2 * np.log(1 - p[hi]))
        out[hi] = -((((((c[0] * q + c[1]) * q + c[2]) * q + c[3]) * q + c[4]) * q + c[5])
                    / ((((d[0] * q + d[1]) * q + d[2]) * q + d[3]) * q + 1))
    if mid.any():
        q = p[mid] - 0.5
        r = q * q
        out[mid] = ((((((a[0] * r + a[1]) * r + a[2]) * r + a[3]) * r + a[4]) * r + a[5]) * q
                    / (((((b[0] * r + b[1]) * r + b[2]) * r + b[3]) * r + b[4]) * r + 1))
    return out


def _norm_pdf(x):
    return np.exp(-0.5 * x * x) / np.sqrt(2 * np.pi)


def _threshold_prior(sq, k, margin=4.5):
    q = np.arange(sq)
    n = q + 1.0
    p = np.minimum(k / n, 1.0)
    pc = np.clip(p, 1e-12, 1 - 1e-12)
    t_star = _norm_ppf(1 - pc)
    sd = np.sqrt(pc * (1 - pc) / n) / np.maximum(_norm_pdf(t_star), 1e-12)
    T = t_star - margin * sd
    T[n <= k + 64] = -1.0e6
    return T.astype(np.float32)


# ----------------------------------------------------------------------
# device kernel
# ----------------------------------------------------------------------
def _build(nc):
    from contextlib import ExitStack
    import concourse.tile as tile
    import concourse.mybir as mybir

    F32, I32, U32 = mybir.dt.float32, mybir.dt.int32, mybir.dt.uint32
    I16, U16, U8 = mybir.dt.int16, mybir.dt.uint16, mybir.dt.uint8
    Op = mybir.AluOpType
    n_tiles = R // 128
    n_chunks = SK // 1024

    x_d = nc.dram_tensor("x", [R, SK], F32, kind="ExternalInput")
    qv_d = nc.dram_tensor("qv", [R, 1], F32, kind="ExternalInput")
    tv_d = nc.dram_tensor("tv", [R, 1], F32, kind="ExternalInput")
    mask_d = nc.dram_tensor("mask", [R, SK], U8, kind="ExternalOutput")
    idx_d = nc.dram_tensor("idx", [R, K], I32, kind="ExternalOutput")
    cnt_d = nc.dram_tensor("cnt", [R, 1], F32, kind="ExternalOutput")

    with tile.TileContext(nc) as tc, ExitStack() as ctx:
        cpool = ctx.enter_context(tc.tile_pool(name="const", bufs=1))
        iota_f = cpool.tile([128, SK], F32)
        nc.gpsimd.iota(iota_f[:], pattern=[[1, SK]], base=0, channel_multiplier=0,
                       allow_small_or_imprecise_dtypes=True)
        iota16 = cpool.tile([128, SK], I16)
        nc.gpsimd.iota(iota16[:], pattern=[[1, SK]], base=0, channel_multiplier=0)
        slotc_f = cpool.tile([128, CAP], F32)
        nc.gpsimd.iota(slotc_f[:], pattern=[[1, CAP]], base=1, channel_multiplier=0,
                       allow_small_or_imprecise_dtypes=True)
        slotk_f = cpool.tile([128, K], F32)
        nc.gpsimd.iota(slotk_f[:], pattern=[[1, K]], base=1, channel_multiplier=0,
                       allow_small_or_imprecise_dtypes=True)
        slotk_u16 = cpool.tile([128, K], U16)
        nc.gpsimd.iota(slotk_u16[:], pattern=[[1, K]], base=0, channel_multiplier=0)
        slot1_u16 = cpool.tile([128, K], U16)
        nc.gpsimd.iota(slot1_u16[:], pattern=[[1, K]], base=1, channel_multiplier=0)
        ones16 = cpool.tile([128, K], U16)
        nc.vector.memset(ones16[:], 1)
        negbig = cpool.tile([128, CAP], F32)
        nc.vector.memset(negbig[:], NEGBIG)
        Act = mybir.ActivationFunctionType
        bc = {}
        for name, val in [("b0", 0.0), ("b1", 1.0), ("bm1", -1.0), ("bm2", -2.0),
                          ("s1", 1.0), ("s2", 2.0), ("sm1", -1.0)]:
            tl = cpool.tile([128, 1], F32, name=f"bc_{name}")
            nc.vector.memset(tl[:], val)
            bc[name] = tl

        xpool = ctx.enter_context(tc.tile_pool(name="x", bufs=2))
        big = ctx.enter_context(tc.tile_pool(name="big", bufs=1))
        sm = ctx.enter_context(tc.tile_pool(name="small", bufs=2))
        opool = ctx.enter_context(tc.tile_pool(name="outs", bufs=2))

        for t in range(n_tiles):
            rows = slice(t * 128, (t + 1) * 128)
            xt = xpool.tile([128, SK], F32)
            nc.sync.dma_start(xt[:], x_d.ap()[rows, :])
            qv = sm.tile([128, 1], F32)
            nc.sync.dma_start(qv[:], qv_d.ap()[rows, :])
            tv = sm.tile([128, 1], F32)
            nc.sync.dma_start(tv[:], tv_d.ap()[rows, :])

            # candidate predicate, prefix positions
            cmpf = big.tile([128, SK], F32)
            nc.vector.tensor_tensor(cmpf[:], iota_f[:],
                                    qv[:].to_broadcast([128, SK]), op=Op.is_le)
            P = big.tile([128, SK], F32)
            nc.vector.scalar_tensor_tensor(P[:], xt[:], tv[:], cmpf[:],
                                           op0=Op.is_gt, op1=Op.logical_and)
            S = big.tile([128, SK], F32)
            nc.vector.tensor_tensor_scan(S[:], P[:], P[:], 0.0,
                                         op0=Op.add, op1=Op.bypass)
            cntt = sm.tile([128, 1], F32)
            nc.scalar.copy(cntt[:], S[:, SK - 1 : SK])
            nc.sync.dma_start(cnt_d.ap()[rows, :], cntt[:])

            nc.vector.tensor_tensor(P[:], S[:], P[:], op=Op.mult)
            nc.vector.scalar_tensor_tensor(P[:], P[:], float(CAP), P[:],
                                           op0=Op.is_le, op1=Op.mult)
            pos16 = big.tile([128, SK], I16)
            nc.vector.tensor_scalar(pos16[:], P[:], -1.0, None, op0=Op.add)
            vidx = big.tile([128, 2 * SK], I16)
            vv = vidx[:].rearrange("p (j t) -> p j t", t=2)
            nc.vector.tensor_scalar(vv[:, :, 0], P[:], 2.0, -2.0,
                                    op0=Op.mult, op1=Op.add)
            nc.vector.tensor_scalar(vv[:, :, 1], P[:], 2.0, -1.0,
                                    op0=Op.mult, op1=Op.add)

            # compaction
            jc = sm.tile([128, CAP], U16)
            nc.gpsimd.local_scatter(jc[:], iota16[:], pos16[:],
                                    channels=128, num_elems=CAP, num_idxs=SK)
            vbuf = sm.tile([128, CAP], F32)
            nc.gpsimd.local_scatter(vbuf[:].bitcast(U16), xt[:].bitcast(U16),
                                    vidx[:], channels=128, num_elems=2 * CAP,
                                    num_idxs=2 * SK)
            padm = sm.tile([128, CAP], U8)
            nc.vector.tensor_tensor(padm[:], slotc_f[:],
                                    cntt[:].to_broadcast([128, CAP]), op=Op.is_gt)
            nc.vector.copy_predicated(vbuf[:], padm[:], negbig[:])

            # extraction
            vals = sm.tile([128, K], F32)
            posb = sm.tile([128, K], U32)
            for i in range(K // 8):
                c8 = slice(i * 8, (i + 1) * 8)
                nc.vector.max(vals[:, c8], vbuf[:])
                nc.vector.max_index(posb[:, c8], vals[:, c8], vbuf[:])
                nc.vector.match_replace(out=vbuf[:], in_to_replace=vals[:, c8],
                                        in_values=vbuf[:], imm_value=SENTINEL)

            # rank -> original column index
            posb16 = sm.tile([128, K], I16)
            nc.scalar.copy(posb16[:], posb[:])
            inv = sm.tile([128, CAP], U16)
            nc.gpsimd.local_scatter(inv[:], slot1_u16[:], posb16[:],
                                    channels=128, num_elems=CAP, num_idxs=K)
            idxs2 = sm.tile([128, CAP], I16)
            nc.vector.tensor_scalar(idxs2[:], inv[:], -1.0, None, op0=Op.add)
            outj = sm.tile([128, K], U16)
            nc.gpsimd.local_scatter(outj[:], jc[:], idxs2[:],
                                    channels=128, num_elems=K, num_idxs=CAP)
            tailm = sm.tile([128, K], U8)
            nc.vector.tensor_tensor(tailm[:], slotk_f[:],
                                    cntt[:].to_broadcast([128, K]), op=Op.is_gt)
            nc.vector.copy_predicated(outj[:], tailm[:], slotk_u16[:])
            idxt = opool.tile([128, K], I32)
            nc.scalar.copy(idxt[:], outj[:])
            nc.sync.dma_start(idx_d.ap()[rows, :], idxt[:])

            # mask
            mask16 = big.tile([128, n_chunks * 1024], U16)
            cidx = sm.tile([128, K], I16)
            g = sm.tile([128, K], F32)
            c = sm.tile([128, K], F32)
            for m in range(n_chunks):
                nc.vector.tensor_scalar(g[:], outj[:], float(1024 * (m + 1)), 4096.0,
                                        op0=Op.is_ge, op1=Op.mult)
                nc.vector.tensor_scalar(c[:], outj[:], float(-1024 * m), None,
                                        op0=Op.add)
                nc.vector.tensor_tensor(cidx[:], c[:], g[:], op=Op.subtract)
                nc.gpsimd.local_scatter(mask16[:, m * 1024 : (m + 1) * 1024],
                                        ones16[:], cidx[:],
                                        channels=128, num_elems=1024, num_idxs=K)
            maskt = opool.tile([128, SK], U8)
            nc.scalar.copy(maskt[:], mask16[:, :SK])
            nc.sync.dma_start(mask_d.ap()[rows, :], maskt[:])

    return nc


def _get_compiled():
    if "nc" not in _CACHE:
        from concourse import bacc

        nc = bacc.Bacc("TRN2", target_bir_lowering=False, debug=False)
        _build(nc)
        nc.compile()
        _CACHE["nc"] = nc
    return _CACHE["nc"]


# ----------------------------------------------------------------------
# exact host fallback for rows where the prior failed (never expected)
# ----------------------------------------------------------------------
def _host_row(xrow, q, k):
    scores = xrow.copy()
    scores[q + 1 :] = np.float32(-1e9)
    idx = np.argsort(-scores, kind="stable")[:k].astype(np.int32)
    mask = np.zeros(xrow.shape[0], dtype=np.uint8)
    mask[idx] = 1
    return mask, idx


def _host_full(index_scores, top_k):
    b, sq, sk = index_scores.shape
    causal = np.triu(np.ones((sq, sk), dtype=bool), k=1)
    scores = np.where(causal[None], np.float32(-1e9), index_scores)
    kk = min(int(top_k), sk)
    idx = np.argsort(-scores, axis=-1, kind="stable")[:, :, :kk].astype(np.int32)
    mask = np.zeros((b, sq, sk), dtype=bool)
    bb = np.arange(b)[:, None, None]
    qq = np.arange(sq)[None, :, None]
    mask[bb, qq, idx] = True
    sparsity = np.float32(1.0) - np.float32(mask.sum()) / np.float32(mask.size)
    return mask, idx, sparsity


# ----------------------------------------------------------------------
# entry point
# ----------------------------------------------------------------------
def kernel(index_scores, top_k):
    index_scores = np.asarray(index_scores, dtype=np.float32)
    if index_scores.shape != (B, SQ, SK) or int(top_k) != K:
        return _host_full(index_scores, int(top_k))

    from concourse.bass_utils import run_bass_kernel_spmd

    nc = _get_compiled()
    T = _threshold_prior(SQ, K)
    in_maps = []
    for c in range(NCORES):
        xs = np.ascontiguousarray(
            index_scores[:, c * QCHUNK : (c + 1) * QCHUNK, :]
        ).reshape(R, SK)
        q = np.tile(np.arange(c * QCHUNK, (c + 1) * QCHUNK, dtype=np.float32), B)
        tvs = np.tile(T[c * QCHUNK : (c + 1) * QCHUNK], B)
        in_maps.append({
            "x": xs,
            "qv": q[:, None].astype(np.float32),
            "tv": tvs[:, None].astype(np.float32),
        })

    res = run_bass_kernel_spmd(nc, in_maps, core_ids=list(range(NCORES)))

    mask = np.empty((B, SQ, SK), dtype=np.uint8)
    idx = np.empty((B, SQ, K), dtype=np.int32)
    cnt = np.empty((B, SQ), dtype=np.float32)
    for c, r in enumerate(res.results):
        qs = slice(c * QCHUNK, (c + 1) * QCHUNK)
        mask[:, qs, :] = r["mask"].reshape(B, QCHUNK, SK)
        idx[:, qs, :] = r["idx"].reshape(B, QCHUNK, K)
        cnt[:, qs] = r["cnt"].reshape(B, QCHUNK)

    qarr = np.arange(SQ, dtype=np.float32)[None, :]
    valid = ((cnt >= K) | (cnt == qarr + 1)) & (cnt <= CAP)
    if not valid.all():
        for bb, qq in np.argwhere(~valid):
            mrow, irow = _host_row(index_scores[bb, qq], int(qq), K)
            mask[bb, qq] = mrow
            idx[bb, qq] = irow

    maskb = mask.astype(bool)
    sparsity = np.float32(1.0) - np.float32(maskb.sum()) / np.float32(maskb.size)
    return maskb, idx, sparsity


# revision 13
# speedup vs baseline: 1.0472x; 1.0472x over previous
"""Trainium2 Bass kernel for nn_AdaptiveTopKSelector (causal top-k masking).

kernel(index_scores [4,4096,4096] f32, top_k=512) ->
    (top_k_mask [4,4096,4096] bool, top_k_indices [4,4096,512] int32,
     sparsity f32 scalar)

Strategy (8 NeuronCores, sequence-parallel over seq_len_q):
  Each core takes all 4 batches x a contiguous 512-wide q-chunk
  (2048 rows of 4096 scores). Per 128-row tile:
    1. candidate predicate P = (j <= q) & (x > T[q]) where T[q] is a
       host-side statistical prior (function of row position only) chosen
       so that K <= #candidates <= CAP with ~4.5 sigma margin;
    2. prefix-sum positions + gpsimd local_scatter compact the candidate
       values (f32 moved as u16 pairs) and their column indices into
       CAP-sized buffers;
    3. K/8 rounds of max8 / max_index / match_replace extract the top-K
       values in exact descending order (ties resolved to the lower
       index by the ascending-scan semantics of max_index/match_replace,
       matching jax.lax.top_k);
    4. two more local_scatters invert the rank->buffer-slot permutation
       into top_k_indices; short rows (q+1 <= K) get their deterministic
       [q+1..K) tail filled from an iota;
    5. the boolean mask is scattered from the final indices in 1024-wide
       chunks.
  A per-row candidate count is exported; any row whose count falls
  outside [K, CAP] (~25 of 16384 rows at this margin/CAP) is recomputed
  exactly on the host, so the result is exact regardless of the prior.
"""

import numpy as np

B, SQ, SK, K, CAP = 4, 4096, 4096, 512, 656
NCORES = 8
QCHUNK = SQ // NCORES
R = B * QCHUNK

SENTINEL = -4.0e9
NEGBIG = -3.0e9

_CACHE = {}


# ----------------------------------------------------------------------
# host-side threshold prior
# ----------------------------------------------------------------------
def _norm_ppf(p):
    """Acklam's inverse normal CDF approximation (|rel err| < 1.2e-9)."""
    p = np.asarray(p, dtype=np.float64)
    a = [-3.969683028665376e+01, 2.209460984245205e+02, -2.759285104469687e+02,
         1.383577518672690e+02, -3.066479806614716e+01, 2.506628277459239e+00]
    b = [-5.447609879822406e+01, 1.615858368580409e+02, -1.556989798598866e+02,
         6.680131188771972e+01, -1.328068155288572e+01]
    c = [-7.784894002430293e-03, -3.223964580411365e-01, -2.400758277161838e+00,
         -2.549732539343734e+00, 4.374664141464968e+00, 2.938163982698783e+00]
    d = [7.784695709041462e-03, 3.224671290700398e-01, 2.445134137142996e+00,
         3.754408661907416e+00]
    out = np.empty_like(p)
    plow, phigh = 0.02425, 1 - 0.02425
    lo = p < plow
    hi = p > phigh
    mid = ~(lo | hi)
    if lo.any():
        q = np.sqrt(-2 * np.log(p[lo]))
        out[lo] = ((((((c[0] * q + c[1]) * q + c[2]) * q + c[3]) * q + c[4]) * q + c[5])
                   / ((((d[0] * q + d[1]) * q + d[2]) * q + d[3]) * q + 1))
    if hi.any():
        q = np.sqrt(-2 * np.log(1 - p[hi]))
        out[hi] = -((((((c[0] * q + c[1]) * q + c[2]) * q + c[3]) * q + c[4]) * q + c[5])
                    / ((((d[0] * q + d[1]) * q + d[2]) * q + d[3]) * q + 1))
    if mid.any():
        q = p[mid] - 0.5
        r = q * q
        out[mid] = ((((((a[0] * r + a[1]) * r + a[2]) * r + a[3]) * r + a[4]) * r + a[5]) * q
                    / (((((b[0] * r + b[1]) * r + b[2]) * r + b[3]) * r + b[4]) * r + 1))
    return out


def _norm_pdf(x):
    return np.exp(-0.5 * x * x) / np.sqrt(2 * np.pi)


def _threshold_prior(sq, k, margin=4.5):
    q = np.arange(sq)
    n = q + 1.0
    p = np.minimum(k / n, 1.0)
    pc = np.clip(p, 1e-12, 1 - 1e-12)
    t_star = _norm_ppf(1 - pc)
    sd = np.sqrt(pc * (1 - pc) / n) / np.maximum(_norm_pdf(t_star), 1e-12)
    T = t_star - margin * sd
    T[n <= k + 64] = -1.0e6
    return T.astype(np.float32)


# ----------------------------------------------------------------------
# device kernel
# ----------------------------------------------------------------------
def _build(nc):
    from contextlib import ExitStack
    import concourse.tile as tile
    import concourse.mybir as mybir

    F32, I32, U32 = mybir.dt.float32, mybir.dt.int32, mybir.dt.uint32
    I16, U16, U8 = mybir.dt.int16, mybir.dt.uint16, mybir.dt.uint8
    Op = mybir.AluOpType
    n_tiles = R // 128
    n_chunks = SK // 1024

    x_d = nc.dram_tensor("x", [R, SK], F32, kind="ExternalInput")
    cm_d = nc.dram_tensor("cm", [R, SK], U8, kind="ExternalInput")
    tv_d = nc.dram_tensor("tv", [R, 1], F32, kind="ExternalInput")
    mask_d = nc.dram_tensor("mask", [R, SK], U8, kind="ExternalOutput")
    idx_d = nc.dram_tensor("idx", [R, K], I32, kind="ExternalOutput")
    cnt_d = nc.dram_tensor("cnt", [R, 1], F32, kind="ExternalOutput")

    with tile.TileContext(nc) as tc, ExitStack() as ctx:
        cpool = ctx.enter_context(tc.tile_pool(name="const", bufs=1))
        iota16 = cpool.tile([128, SK], I16)
        nc.gpsimd.iota(iota16[:], pattern=[[1, SK]], base=0, channel_multiplier=0)
        slotc_f = cpool.tile([128, CAP], F32)
        nc.gpsimd.iota(slotc_f[:], pattern=[[1, CAP]], base=1, channel_multiplier=0,
                       allow_small_or_imprecise_dtypes=True)
        slotk_f = cpool.tile([128, K], F32)
        nc.gpsimd.iota(slotk_f[:], pattern=[[1, K]], base=1, channel_multiplier=0,
                       allow_small_or_imprecise_dtypes=True)
        slotk_u16 = cpool.tile([128, K], U16)
        nc.gpsimd.iota(slotk_u16[:], pattern=[[1, K]], base=0, channel_multiplier=0)
        slot1_u16 = cpool.tile([128, K], U16)
        nc.gpsimd.iota(slot1_u16[:], pattern=[[1, K]], base=1, channel_multiplier=0)
        ones16 = cpool.tile([128, K], U16)
        nc.vector.memset(ones16[:], 1)
        negbig = cpool.tile([128, CAP], F32)
        nc.vector.memset(negbig[:], NEGBIG)
        Act = mybir.ActivationFunctionType
        bc = {}
        for name, val in [("b0", 0.0), ("b1", 1.0), ("bm1", -1.0), ("bm2", -2.0),
                          ("s1", 1.0), ("s2", 2.0), ("sm1", -1.0)]:
            tl = cpool.tile([128, 1], F32, name=f"bc_{name}")
            nc.vector.memset(tl[:], val)
            bc[name] = tl

        xpool = ctx.enter_context(tc.tile_pool(name="x", bufs=2))
        big = ctx.enter_context(tc.tile_pool(name="big", bufs=1))
        sm = ctx.enter_context(tc.tile_pool(name="small", bufs=2))
        opool = ctx.enter_context(tc.tile_pool(name="outs", bufs=2))

        for t in range(n_tiles):
            rows = slice(t * 128, (t + 1) * 128)
            xt = xpool.tile([128, SK], F32)
            nc.sync.dma_start(xt[:], x_d.ap()[rows, :])
            cmt = sm.tile([128, SK], U8)
            nc.sync.dma_start(cmt[:], cm_d.ap()[rows, :])
            tv = sm.tile([128, 1], F32)
            nc.sync.dma_start(tv[:], tv_d.ap()[rows, :])

            # candidate predicate, prefix positions (causal mask DMA'd: it is
            # a constant function of row position, like the iota tiles)
            P = big.tile([128, SK], F32)
            nc.vector.scalar_tensor_tensor(P[:], xt[:], tv[:], cmt[:],
                                           op0=Op.is_gt, op1=Op.logical_and)
            S = big.tile([128, SK], F32)
            nc.vector.tensor_tensor_scan(S[:], P[:], P[:], 0.0,
                                         op0=Op.add, op1=Op.bypass)
            cntt = sm.tile([128, 1], F32)
            nc.scalar.copy(cntt[:], S[:, SK - 1 : SK])
            nc.sync.dma_start(cnt_d.ap()[rows, :], cntt[:])

            nc.vector.tensor_tensor(P[:], S[:], P[:], op=Op.mult)
            nc.vector.scalar_tensor_tensor(P[:], P[:], float(CAP), P[:],
                                           op0=Op.is_le, op1=Op.mult)
            pos16 = big.tile([128, SK], I16)
            nc.vector.tensor_scalar(pos16[:], P[:], -1.0, None, op0=Op.add)
            vidx = big.tile([128, 2 * SK], I16)
            vv = vidx[:].rearrange("p (j t) -> p j t", t=2)
            nc.vector.tensor_scalar(vv[:, :, 0], P[:], 2.0, -2.0,
                                    op0=Op.mult, op1=Op.add)
            nc.vector.tensor_scalar(vv[:, :, 1], P[:], 2.0, -1.0,
                                    op0=Op.mult, op1=Op.add)

            # compaction
            jc = sm.tile([128, CAP], U16)
            nc.gpsimd.local_scatter(jc[:], iota16[:], pos16[:],
                                    channels=128, num_elems=CAP, num_idxs=SK)
            vbuf = sm.tile([128, CAP], F32)
            nc.gpsimd.local_scatter(vbuf[:].bitcast(U16), xt[:].bitcast(U16),
                                    vidx[:], channels=128, num_elems=2 * CAP,
                                    num_idxs=2 * SK)
            padm = sm.tile([128, CAP], U8)
            nc.vector.tensor_tensor(padm[:], slotc_f[:],
                                    cntt[:].to_broadcast([128, CAP]), op=Op.is_gt)
            nc.vector.copy_predicated(vbuf[:], padm[:], negbig[:])

            # extraction
            vals = sm.tile([128, K], F32)
            posb = sm.tile([128, K], U32)
            for i in range(K // 8):
                c8 = slice(i * 8, (i + 1) * 8)
                nc.vector.max(vals[:, c8], vbuf[:])
                nc.vector.max_index(posb[:, c8], vals[:, c8], vbuf[:])
                if i < K // 8 - 1:
                    nc.vector.match_replace(out=vbuf[:], in_to_replace=vals[:, c8],
                                            in_values=vbuf[:], imm_value=SENTINEL)

            # rank -> original column index
            posb16 = sm.tile([128, K], I16)
            nc.scalar.copy(posb16[:], posb[:])
            inv = sm.tile([128, CAP], U16)
            nc.gpsimd.local_scatter(inv[:], slot1_u16[:], posb16[:],
                                    channels=128, num_elems=CAP, num_idxs=K)
            idxs2 = sm.tile([128, CAP], I16)
            nc.vector.tensor_scalar(idxs2[:], inv[:], -1.0, None, op0=Op.add)
            outj = sm.tile([128, K], U16)
            nc.gpsimd.local_scatter(outj[:], jc[:], idxs2[:],
                                    channels=128, num_elems=K, num_idxs=CAP)
            tailm = sm.tile([128, K], U8)
            nc.vector.tensor_tensor(tailm[:], slotk_f[:],
                                    cntt[:].to_broadcast([128, K]), op=Op.is_gt)
            nc.vector.copy_predicated(outj[:], tailm[:], slotk_u16[:])
            idxt = opool.tile([128, K], I32)
            nc.scalar.copy(idxt[:], outj[:])
            nc.sync.dma_start(idx_d.ap()[rows, :], idxt[:])

            # mask
            mask16 = big.tile([128, n_chunks * 1024], U16)
            cidx = sm.tile([128, K], I16)
            g = sm.tile([128, K], F32)
            for m in range(n_chunks):
                nc.vector.tensor_scalar(g[:], outj[:], float(1024 * (m + 1)), 4096.0,
                                        op0=Op.is_ge, op1=Op.mult)
                nc.vector.scalar_tensor_tensor(cidx[:], outj[:], float(-1024 * m),
                                               g[:], op0=Op.add, op1=Op.subtract)
                nc.gpsimd.local_scatter(mask16[:, m * 1024 : (m + 1) * 1024],
                                        ones16[:], cidx[:],
                                        channels=128, num_elems=1024, num_idxs=K)
            maskt = opool.tile([128, SK], U8)
            nc.scalar.copy(maskt[:], mask16[:, :SK])
            nc.sync.dma_start(mask_d.ap()[rows, :], maskt[:])

    return nc


def _get_compiled():
    if "nc" not in _CACHE:
        from concourse import bacc

        nc = bacc.Bacc("TRN2", target_bir_lowering=False, debug=False)
        _build(nc)
        nc.compile()
        _CACHE["nc"] = nc
    return _CACHE["nc"]


# ----------------------------------------------------------------------
# exact host fallback for rows where the prior failed (never expected)
# ----------------------------------------------------------------------
def _host_row(xrow, q, k):
    scores = xrow.copy()
    scores[q + 1 :] = np.float32(-1e9)
    idx = np.argsort(-scores, kind="stable")[:k].astype(np.int32)
    mask = np.zeros(xrow.shape[0], dtype=np.uint8)
    mask[idx] = 1
    return mask, idx


def _host_full(index_scores, top_k):
    b, sq, sk = index_scores.shape
    causal = np.triu(np.ones((sq, sk), dtype=bool), k=1)
    scores = np.where(causal[None], np.float32(-1e9), index_scores)
    kk = min(int(top_k), sk)
    idx = np.argsort(-scores, axis=-1, kind="stable")[:, :, :kk].astype(np.int32)
    mask = np.zeros((b, sq, sk), dtype=bool)
    bb = np.arange(b)[:, None, None]
    qq = np.arange(sq)[None, :, None]
    mask[bb, qq, idx] = True
    sparsity = np.float32(1.0) - np.float32(mask.sum()) / np.float32(mask.size)
    return mask, idx, sparsity


# ----------------------------------------------------------------------
# entry point
# ----------------------------------------------------------------------
def kernel(index_scores, top_k):
    index_scores = np.asarray(index_scores, dtype=np.float32)
    if index_scores.shape != (B, SQ, SK) or int(top_k) != K:
        return _host_full(index_scores, int(top_k))

    from concourse.bass_utils import run_bass_kernel_spmd

    nc = _get_compiled()
    T = _threshold_prior(SQ, K)
    in_maps = []
    for c in range(NCORES):
        xs = np.ascontiguousarray(
            index_scores[:, c * QCHUNK : (c + 1) * QCHUNK, :]
        ).reshape(R, SK)
        q = np.tile(np.arange(c * QCHUNK, (c + 1) * QCHUNK, dtype=np.int32), B)
        cm = (np.arange(SK, dtype=np.int32)[None, :] <= q[:, None]).astype(np.uint8)
        tvs = np.tile(T[c * QCHUNK : (c + 1) * QCHUNK], B)
        in_maps.append({
            "x": xs,
            "cm": cm,
            "tv": tvs[:, None].astype(np.float32),
        })

    res = run_bass_kernel_spmd(nc, in_maps, core_ids=list(range(NCORES)))

    mask = np.empty((B, SQ, SK), dtype=np.uint8)
    idx = np.empty((B, SQ, K), dtype=np.int32)
    cnt = np.empty((B, SQ), dtype=np.float32)
    for c, r in enumerate(res.results):
        qs = slice(c * QCHUNK, (c + 1) * QCHUNK)
        mask[:, qs, :] = r["mask"].reshape(B, QCHUNK, SK)
        idx[:, qs, :] = r["idx"].reshape(B, QCHUNK, K)
        cnt[:, qs] = r["cnt"].reshape(B, QCHUNK)

    qarr = np.arange(SQ, dtype=np.float32)[None, :]
    valid = ((cnt >= K) | (cnt == qarr + 1)) & (cnt <= CAP)
    if not valid.all():
        for bb, qq in np.argwhere(~valid):
            mrow, irow = _host_row(index_scores[bb, qq], int(qq), K)
            mask[bb, qq] = mrow
            idx[bb, qq] = irow

    maskb = mask.astype(bool)
    sparsity = np.float32(1.0) - np.float32(maskb.sum()) / np.float32(maskb.size)
    return maskb, idx, sparsity


# revision 14
# speedup vs baseline: 1.0743x; 1.0259x over previous
"""Trainium2 Bass kernel for nn_AdaptiveTopKSelector (causal top-k masking).

kernel(index_scores [4,4096,4096] f32, top_k=512) ->
    (top_k_mask [4,4096,4096] bool, top_k_indices [4,4096,512] int32,
     sparsity f32 scalar)

Strategy (8 NeuronCores, sequence-parallel over seq_len_q):
  Each core takes all 4 batches x a contiguous 512-wide q-chunk
  (2048 rows of 4096 scores). Per 128-row tile:
    1. candidate predicate P = (j <= q) & (x > T[q]) where T[q] is a
       host-side statistical prior (function of row position only) chosen
       so that K <= #candidates <= CAP with ~4.5 sigma margin;
    2. prefix-sum positions + gpsimd local_scatter compact the candidate
       values (f32 moved as u16 pairs) and their column indices into
       CAP-sized buffers;
    3. K/8 rounds of max8 / max_index / match_replace extract the top-K
       values in exact descending order (ties resolved to the lower
       index by the ascending-scan semantics of max_index/match_replace,
       matching jax.lax.top_k);
    4. two more local_scatters invert the rank->buffer-slot permutation
       into top_k_indices; short rows (q+1 <= K) get their deterministic
       [q+1..K) tail filled from an iota;
    5. the boolean mask is scattered from the final indices in 1024-wide
       chunks.
  A per-row candidate count is exported; any row whose count falls
  outside [K, CAP] (~25 of 16384 rows at this margin/CAP) is recomputed
  exactly on the host, so the result is exact regardless of the prior.
"""

import numpy as np

B, SQ, SK, K, CAP = 4, 4096, 4096, 512, 656
NCORES = 8
QCHUNK = SQ // NCORES
R = B * QCHUNK

SENTINEL = -4.0e9
NEGBIG = -3.0e9

_CACHE = {}


# ----------------------------------------------------------------------
# host-side threshold prior
# ----------------------------------------------------------------------
def _norm_ppf(p):
    """Acklam's inverse normal CDF approximation (|rel err| < 1.2e-9)."""
    p = np.asarray(p, dtype=np.float64)
    a = [-3.969683028665376e+01, 2.209460984245205e+02, -2.759285104469687e+02,
         1.383577518672690e+02, -3.066479806614716e+01, 2.506628277459239e+00]
    b = [-5.447609879822406e+01, 1.615858368580409e+02, -1.556989798598866e+02,
         6.680131188771972e+01, -1.328068155288572e+01]
    c = [-7.784894002430293e-03, -3.223964580411365e-01, -2.400758277161838e+00,
         -2.549732539343734e+00, 4.374664141464968e+00, 2.938163982698783e+00]
    d = [7.784695709041462e-03, 3.224671290700398e-01, 2.445134137142996e+00,
         3.754408661907416e+00]
    out = np.empty_like(p)
    plow, phigh = 0.02425, 1 - 0.02425
    lo = p < plow
    hi = p > phigh
    mid = ~(lo | hi)
    if lo.any():
        q = np.sqrt(-2 * np.log(p[lo]))
        out[lo] = ((((((c[0] * q + c[1]) * q + c[2]) * q + c[3]) * q + c[4]) * q + c[5])
                   / ((((d[0] * q + d[1]) * q + d[2]) * q + d[3]) * q + 1))
    if hi.any():
        q = np.sqrt(-2 * np.log(1 - p[hi]))
        out[hi] = -((((((c[0] * q + c[1]) * q + c[2]) * q + c[3]) * q + c[4]) * q + c[5])
                    / ((((d[0] * q + d[1]) * q + d[2]) * q + d[3]) * q + 1))
    if mid.any():
        q = p[mid] - 0.5
        r = q * q
        out[mid] = ((((((a[0] * r + a[1]) * r + a[2]) * r + a[3]) * r + a[4]) * r + a[5]) * q
                    / (((((b[0] * r + b[1]) * r + b[2]) * r + b[3]) * r + b[4]) * r + 1))
    return out


def _norm_pdf(x):
    return np.exp(-0.5 * x * x) / np.sqrt(2 * np.pi)


def _threshold_prior(sq, k, margin=4.5):
    q = np.arange(sq)
    n = q + 1.0
    p = np.minimum(k / n, 1.0)
    pc = np.clip(p, 1e-12, 1 - 1e-12)
    t_star = _norm_ppf(1 - pc)
    sd = np.sqrt(pc * (1 - pc) / n) / np.maximum(_norm_pdf(t_star), 1e-12)
    T = t_star - margin * sd
    T[n <= k + 64] = -1.0e6
    return T.astype(np.float32)


# ----------------------------------------------------------------------
# device kernel
# ----------------------------------------------------------------------
def _build(nc):
    from contextlib import ExitStack
    import concourse.tile as tile
    import concourse.mybir as mybir

    F32, I32, U32 = mybir.dt.float32, mybir.dt.int32, mybir.dt.uint32
    I16, U16, U8 = mybir.dt.int16, mybir.dt.uint16, mybir.dt.uint8
    Op = mybir.AluOpType
    n_tiles = R // 128
    n_chunks = SK // 1024

    x_d = nc.dram_tensor("x", [R, SK], F32, kind="ExternalInput")
    cm_d = nc.dram_tensor("cm", [R, SK], U8, kind="ExternalInput")
    tv_d = nc.dram_tensor("tv", [R, 1], F32, kind="ExternalInput")
    mask_d = nc.dram_tensor("mask", [R, SK], U8, kind="ExternalOutput")
    idx_d = nc.dram_tensor("idx", [R, K], I32, kind="ExternalOutput")
    cnt_d = nc.dram_tensor("cnt", [R, 1], F32, kind="ExternalOutput")

    with tile.TileContext(nc) as tc, ExitStack() as ctx:
        cpool = ctx.enter_context(tc.tile_pool(name="const", bufs=1))
        iota16 = cpool.tile([128, SK], I16)
        nc.gpsimd.iota(iota16[:], pattern=[[1, SK]], base=0, channel_multiplier=0)
        slotc_f = cpool.tile([128, CAP], F32)
        nc.gpsimd.iota(slotc_f[:], pattern=[[1, CAP]], base=1, channel_multiplier=0,
                       allow_small_or_imprecise_dtypes=True)
        slotk_u16 = cpool.tile([128, K], U16)
        nc.gpsimd.iota(slotk_u16[:], pattern=[[1, K]], base=0, channel_multiplier=0)
        slot1_u16 = cpool.tile([128, K], U16)
        nc.gpsimd.iota(slot1_u16[:], pattern=[[1, K]], base=1, channel_multiplier=0)
        ones16 = cpool.tile([128, K], U16)
        nc.vector.memset(ones16[:], 1)
        negbig = cpool.tile([128, CAP], F32)
        nc.vector.memset(negbig[:], NEGBIG)
        Act = mybir.ActivationFunctionType
        bc = {}
        for name, val in [("b0", 0.0), ("b1", 1.0), ("bm1", -1.0), ("bm2", -2.0),
                          ("s1", 1.0), ("s2", 2.0), ("sm1", -1.0)]:
            tl = cpool.tile([128, 1], F32, name=f"bc_{name}")
            nc.vector.memset(tl[:], val)
            bc[name] = tl

        xpool = ctx.enter_context(tc.tile_pool(name="x", bufs=2))
        ps = ctx.enter_context(tc.tile_pool(name="ps", bufs=2))
        big = ctx.enter_context(tc.tile_pool(name="big", bufs=1))
        sm = ctx.enter_context(tc.tile_pool(name="small", bufs=2))
        opool = ctx.enter_context(tc.tile_pool(name="outs", bufs=2))

        for t in range(n_tiles):
            rows = slice(t * 128, (t + 1) * 128)
            xt = xpool.tile([128, SK], F32)
            nc.sync.dma_start(xt[:], x_d.ap()[rows, :])
            cmt = sm.tile([128, SK], U8)
            nc.sync.dma_start(cmt[:], cm_d.ap()[rows, :])
            tv = sm.tile([128, 1], F32)
            nc.sync.dma_start(tv[:], tv_d.ap()[rows, :])

            # candidate predicate, prefix positions (causal mask DMA'd: it is
            # a constant function of row position, like the iota tiles)
            P = ps.tile([128, SK], F32)
            nc.vector.scalar_tensor_tensor(P[:], xt[:], tv[:], cmt[:],
                                           op0=Op.is_gt, op1=Op.logical_and)
            S = ps.tile([128, SK], F32)
            nc.vector.tensor_tensor_scan(S[:], P[:], P[:], 0.0,
                                         op0=Op.add, op1=Op.bypass)
            cntt = sm.tile([128, 1], F32)
            nc.scalar.copy(cntt[:], S[:, SK - 1 : SK])
            nc.sync.dma_start(cnt_d.ap()[rows, :], cntt[:])

            nc.vector.tensor_tensor(P[:], S[:], P[:], op=Op.mult)
            nc.vector.scalar_tensor_tensor(P[:], P[:], float(CAP), P[:],
                                           op0=Op.is_le, op1=Op.mult)
            pos16 = big.tile([128, SK], I16)
            nc.scalar.activation(pos16[:], P[:], Act.Identity,
                                 bias=bc["bm1"][:], scale=bc["s1"][:])
            vidx = big.tile([128, 2 * SK], I16)
            vv = vidx[:].rearrange("p (j t) -> p j t", t=2)
            nc.scalar.activation(vv[:, :, 0], P[:], Act.Identity,
                                 bias=bc["bm2"][:], scale=bc["s2"][:])
            nc.scalar.activation(vv[:, :, 1], P[:], Act.Identity,
                                 bias=bc["bm1"][:], scale=bc["s2"][:])

            # compaction
            jc = sm.tile([128, CAP], U16)
            nc.gpsimd.local_scatter(jc[:], iota16[:], pos16[:],
                                    channels=128, num_elems=CAP, num_idxs=SK)
            vbuf = sm.tile([128, CAP], F32)
            nc.gpsimd.local_scatter(vbuf[:].bitcast(U16), xt[:].bitcast(U16),
                                    vidx[:], channels=128, num_elems=2 * CAP,
                                    num_idxs=2 * SK)
            padm = sm.tile([128, CAP], U8)
            nc.vector.tensor_tensor(padm[:], slotc_f[:],
                                    cntt[:].to_broadcast([128, CAP]), op=Op.is_gt)
            nc.vector.copy_predicated(vbuf[:], padm[:], negbig[:])

            # extraction
            vals = sm.tile([128, K], F32)
            posb = sm.tile([128, K], U32)
            for i in range(K // 8):
                c8 = slice(i * 8, (i + 1) * 8)
                nc.vector.max(vals[:, c8], vbuf[:])
                nc.vector.max_index(posb[:, c8], vals[:, c8], vbuf[:])
                if i < K // 8 - 1:
                    nc.vector.match_replace(out=vbuf[:], in_to_replace=vals[:, c8],
                                            in_values=vbuf[:], imm_value=SENTINEL)

            # rank -> original column index
            posb16 = sm.tile([128, K], I16)
            nc.scalar.copy(posb16[:], posb[:])
            inv = sm.tile([128, CAP], U16)
            nc.gpsimd.local_scatter(inv[:], slot1_u16[:], posb16[:],
                                    channels=128, num_elems=CAP, num_idxs=K)
            idxs2 = sm.tile([128, CAP], I16)
            nc.scalar.activation(idxs2[:], inv[:], Act.Identity,
                                 bias=bc["bm1"][:], scale=bc["s1"][:])
            outj = sm.tile([128, K], U16)
            nc.gpsimd.local_scatter(outj[:], jc[:], idxs2[:],
                                    channels=128, num_elems=K, num_idxs=CAP)
            tailm = sm.tile([128, K], U8)
            nc.vector.tensor_tensor(tailm[:], slotc_f[:, :K],
                                    cntt[:].to_broadcast([128, K]), op=Op.is_gt)
            nc.vector.copy_predicated(outj[:], tailm[:], slotk_u16[:])
            idxt = opool.tile([128, K], I32)
            nc.scalar.copy(idxt[:], outj[:])
            nc.sync.dma_start(idx_d.ap()[rows, :], idxt[:])

            # mask
            mask16 = big.tile([128, n_chunks * 1024], U16)
            cidx = sm.tile([128, K], I16)
            g = sm.tile([128, K], F32)
            for m in range(n_chunks):
                nc.vector.tensor_scalar(g[:], outj[:], float(1024 * (m + 1)), 4096.0,
                                        op0=Op.is_ge, op1=Op.mult)
                nc.vector.scalar_tensor_tensor(cidx[:], outj[:], float(-1024 * m),
                                               g[:], op0=Op.add, op1=Op.subtract)
                nc.gpsimd.local_scatter(mask16[:, m * 1024 : (m + 1) * 1024],
                                        ones16[:], cidx[:],
                                        channels=128, num_elems=1024, num_idxs=K)
            maskt = opool.tile([128, SK], U8)
            nc.scalar.copy(maskt[:], mask16[:, :SK])
            nc.sync.dma_start(mask_d.ap()[rows, :], maskt[:])

    return nc


def _get_compiled():
    if "nc" not in _CACHE:
        from concourse import bacc

        nc = bacc.Bacc("TRN2", target_bir_lowering=False, debug=False)
        _build(nc)
        nc.compile()
        _CACHE["nc"] = nc
    return _CACHE["nc"]


# ----------------------------------------------------------------------
# exact host fallback for rows where the prior failed (never expected)
# ----------------------------------------------------------------------
def _host_row(xrow, q, k):
    scores = xrow.copy()
    scores[q + 1 :] = np.float32(-1e9)
    idx = np.argsort(-scores, kind="stable")[:k].astype(np.int32)
    mask = np.zeros(xrow.shape[0], dtype=np.uint8)
    mask[idx] = 1
    return mask, idx


def _host_full(index_scores, top_k):
    b, sq, sk = index_scores.shape
    causal = np.triu(np.ones((sq, sk), dtype=bool), k=1)
    scores = np.where(causal[None], np.float32(-1e9), index_scores)
    kk = min(int(top_k), sk)
    idx = np.argsort(-scores, axis=-1, kind="stable")[:, :, :kk].astype(np.int32)
    mask = np.zeros((b, sq, sk), dtype=bool)
    bb = np.arange(b)[:, None, None]
    qq = np.arange(sq)[None, :, None]
    mask[bb, qq, idx] = True
    sparsity = np.float32(1.0) - np.float32(mask.sum()) / np.float32(mask.size)
    return mask, idx, sparsity


# ----------------------------------------------------------------------
# entry point
# ----------------------------------------------------------------------
def kernel(index_scores, top_k):
    index_scores = np.asarray(index_scores, dtype=np.float32)
    if index_scores.shape != (B, SQ, SK) or int(top_k) != K:
        return _host_full(index_scores, int(top_k))

    from concourse.bass_utils import run_bass_kernel_spmd

    nc = _get_compiled()
    T = _threshold_prior(SQ, K)
    in_maps = []
    for c in range(NCORES):
        xs = np.ascontiguousarray(
            index_scores[:, c * QCHUNK : (c + 1) * QCHUNK, :]
        ).reshape(R, SK)
        q = np.tile(np.arange(c * QCHUNK, (c + 1) * QCHUNK, dtype=np.int32), B)
        cm = (np.arange(SK, dtype=np.int32)[None, :] <= q[:, None]).astype(np.uint8)
        tvs = np.tile(T[c * QCHUNK : (c + 1) * QCHUNK], B)
        in_maps.append({
            "x": xs,
            "cm": cm,
            "tv": tvs[:, None].astype(np.float32),
        })

    res = run_bass_kernel_spmd(nc, in_maps, core_ids=list(range(NCORES)))

    mask = np.empty((B, SQ, SK), dtype=np.uint8)
    idx = np.empty((B, SQ, K), dtype=np.int32)
    cnt = np.empty((B, SQ), dtype=np.float32)
    for c, r in enumerate(res.results):
        qs = slice(c * QCHUNK, (c + 1) * QCHUNK)
        mask[:, qs, :] = r["mask"].reshape(B, QCHUNK, SK)
        idx[:, qs, :] = r["idx"].reshape(B, QCHUNK, K)
        cnt[:, qs] = r["cnt"].reshape(B, QCHUNK)

    qarr = np.arange(SQ, dtype=np.float32)[None, :]
    valid = ((cnt >= K) | (cnt == qarr + 1)) & (cnt <= CAP)
    if not valid.all():
        for bb, qq in np.argwhere(~valid):
            mrow, irow = _host_row(index_scores[bb, qq], int(qq), K)
            mask[bb, qq] = mrow
            idx[bb, qq] = irow

    maskb = mask.astype(bool)
    sparsity = np.float32(1.0) - np.float32(maskb.sum()) / np.float32(maskb.size)
    return maskb, idx, sparsity


# revision 15
# speedup vs baseline: 1.1074x; 1.0308x over previous
"""Trainium2 Bass kernel for nn_AdaptiveTopKSelector (causal top-k masking).

kernel(index_scores [4,4096,4096] f32, top_k=512) ->
    (top_k_mask [4,4096,4096] bool, top_k_indices [4,4096,512] int32,
     sparsity f32 scalar)

Strategy (8 NeuronCores, sequence-parallel over seq_len_q):
  Each core takes all 4 batches x a contiguous 512-wide q-chunk
  (2048 rows of 4096 scores). Per 128-row tile:
    1. candidate predicate P = (j <= q) & (x > T[q]) where T[q] is a
       host-side statistical prior (function of row position only) chosen
       so that K <= #candidates <= CAP with ~4.5 sigma margin;
    2. prefix-sum positions + gpsimd local_scatter compact the candidate
       values (f32 moved as u16 pairs) and their column indices into
       CAP-sized buffers;
    3. K/8 rounds of max8 / max_index / match_replace extract the top-K
       values in exact descending order (ties resolved to the lower
       index by the ascending-scan semantics of max_index/match_replace,
       matching jax.lax.top_k);
    4. two more local_scatters invert the rank->buffer-slot permutation
       into top_k_indices; short rows (q+1 <= K) get their deterministic
       [q+1..K) tail filled from an iota;
    5. the boolean mask is scattered from the final indices in 1024-wide
       chunks.
  A per-row candidate count is exported; any row whose count falls
  outside [K, CAP] (~120 of 16384 rows at this margin/CAP) is recomputed
  exactly on the host, so the result is exact regardless of the prior.
"""

import numpy as np

B, SQ, SK, K, CAP = 4, 4096, 4096, 512, 624
NCORES = 8
QCHUNK = SQ // NCORES
R = B * QCHUNK

SENTINEL = -4.0e9
NEGBIG = -3.0e9

_CACHE = {}


# ----------------------------------------------------------------------
# host-side threshold prior
# ----------------------------------------------------------------------
def _norm_ppf(p):
    """Acklam's inverse normal CDF approximation (|rel err| < 1.2e-9)."""
    p = np.asarray(p, dtype=np.float64)
    a = [-3.969683028665376e+01, 2.209460984245205e+02, -2.759285104469687e+02,
         1.383577518672690e+02, -3.066479806614716e+01, 2.506628277459239e+00]
    b = [-5.447609879822406e+01, 1.615858368580409e+02, -1.556989798598866e+02,
         6.680131188771972e+01, -1.328068155288572e+01]
    c = [-7.784894002430293e-03, -3.223964580411365e-01, -2.400758277161838e+00,
         -2.549732539343734e+00, 4.374664141464968e+00, 2.938163982698783e+00]
    d = [7.784695709041462e-03, 3.224671290700398e-01, 2.445134137142996e+00,
         3.754408661907416e+00]
    out = np.empty_like(p)
    plow, phigh = 0.02425, 1 - 0.02425
    lo = p < plow
    hi = p > phigh
    mid = ~(lo | hi)
    if lo.any():
        q = np.sqrt(-2 * np.log(p[lo]))
        out[lo] = ((((((c[0] * q + c[1]) * q + c[2]) * q + c[3]) * q + c[4]) * q + c[5])
                   / ((((d[0] * q + d[1]) * q + d[2]) * q + d[3]) * q + 1))
    if hi.any():
        q = np.sqrt(-2 * np.log(1 - p[hi]))
        out[hi] = -((((((c[0] * q + c[1]) * q + c[2]) * q + c[3]) * q + c[4]) * q + c[5])
                    / ((((d[0] * q + d[1]) * q + d[2]) * q + d[3]) * q + 1))
    if mid.any():
        q = p[mid] - 0.5
        r = q * q
        out[mid] = ((((((a[0] * r + a[1]) * r + a[2]) * r + a[3]) * r + a[4]) * r + a[5]) * q
                    / (((((b[0] * r + b[1]) * r + b[2]) * r + b[3]) * r + b[4]) * r + 1))
    return out


def _norm_pdf(x):
    return np.exp(-0.5 * x * x) / np.sqrt(2 * np.pi)


def _threshold_prior(sq, k, margin=3.0):
    q = np.arange(sq)
    n = q + 1.0
    p = np.minimum(k / n, 1.0)
    pc = np.clip(p, 1e-12, 1 - 1e-12)
    t_star = _norm_ppf(1 - pc)
    sd = np.sqrt(pc * (1 - pc) / n) / np.maximum(_norm_pdf(t_star), 1e-12)
    T = t_star - margin * sd
    T[n <= k + 64] = -1.0e6
    return T.astype(np.float32)


# ----------------------------------------------------------------------
# device kernel
# ----------------------------------------------------------------------
def _build(nc):
    from contextlib import ExitStack
    import concourse.tile as tile
    import concourse.mybir as mybir

    F32, I32, U32 = mybir.dt.float32, mybir.dt.int32, mybir.dt.uint32
    I16, U16, U8 = mybir.dt.int16, mybir.dt.uint16, mybir.dt.uint8
    Op = mybir.AluOpType
    n_tiles = R // 128
    n_chunks = SK // 1024

    x_d = nc.dram_tensor("x", [R, SK], F32, kind="ExternalInput")
    cm_d = nc.dram_tensor("cm", [R, SK], U8, kind="ExternalInput")
    tv_d = nc.dram_tensor("tv", [R, 1], F32, kind="ExternalInput")
    mask_d = nc.dram_tensor("mask", [R, SK], U8, kind="ExternalOutput")
    idx_d = nc.dram_tensor("idx", [R, K], I32, kind="ExternalOutput")
    cnt_d = nc.dram_tensor("cnt", [R, 1], F32, kind="ExternalOutput")

    with tile.TileContext(nc) as tc, ExitStack() as ctx:
        cpool = ctx.enter_context(tc.tile_pool(name="const", bufs=1))
        iota16 = cpool.tile([128, SK], I16)
        nc.gpsimd.iota(iota16[:], pattern=[[1, SK]], base=0, channel_multiplier=0)
        slotc_f = cpool.tile([128, CAP], F32)
        nc.gpsimd.iota(slotc_f[:], pattern=[[1, CAP]], base=1, channel_multiplier=0,
                       allow_small_or_imprecise_dtypes=True)
        slotk_u16 = cpool.tile([128, K], U16)
        nc.gpsimd.iota(slotk_u16[:], pattern=[[1, K]], base=0, channel_multiplier=0)
        slot1_u16 = cpool.tile([128, K], U16)
        nc.gpsimd.iota(slot1_u16[:], pattern=[[1, K]], base=1, channel_multiplier=0)
        ones16 = cpool.tile([128, K], U16)
        nc.vector.memset(ones16[:], 1)
        negbig = cpool.tile([128, CAP], F32)
        nc.vector.memset(negbig[:], NEGBIG)
        Act = mybir.ActivationFunctionType
        bc = {}
        for name, val in [("b0", 0.0), ("b1", 1.0), ("bm1", -1.0), ("bm2", -2.0),
                          ("s1", 1.0), ("s2", 2.0), ("sm1", -1.0)]:
            tl = cpool.tile([128, 1], F32, name=f"bc_{name}")
            nc.vector.memset(tl[:], val)
            bc[name] = tl

        xpool = ctx.enter_context(tc.tile_pool(name="x", bufs=2))
        ps = ctx.enter_context(tc.tile_pool(name="ps", bufs=2))
        big = ctx.enter_context(tc.tile_pool(name="big", bufs=1))
        sm = ctx.enter_context(tc.tile_pool(name="small", bufs=2))
        opool = ctx.enter_context(tc.tile_pool(name="outs", bufs=2))

        for t in range(n_tiles):
            rows = slice(t * 128, (t + 1) * 128)
            xt = xpool.tile([128, SK], F32)
            nc.sync.dma_start(xt[:], x_d.ap()[rows, :])
            cmt = sm.tile([128, SK], U8)
            nc.sync.dma_start(cmt[:], cm_d.ap()[rows, :])
            tv = sm.tile([128, 1], F32)
            nc.sync.dma_start(tv[:], tv_d.ap()[rows, :])

            # candidate predicate, prefix positions (causal mask DMA'd: it is
            # a constant function of row position, like the iota tiles)
            P = ps.tile([128, SK], F32)
            nc.vector.scalar_tensor_tensor(P[:], xt[:], tv[:], cmt[:],
                                           op0=Op.is_gt, op1=Op.logical_and)
            S = ps.tile([128, SK], F32)
            nc.vector.tensor_tensor_scan(S[:], P[:], P[:], 0.0,
                                         op0=Op.add, op1=Op.bypass)
            cntt = sm.tile([128, 1], F32)
            nc.scalar.copy(cntt[:], S[:, SK - 1 : SK])
            nc.sync.dma_start(cnt_d.ap()[rows, :], cntt[:])

            nc.vector.tensor_tensor(P[:], S[:], P[:], op=Op.mult)
            nc.vector.scalar_tensor_tensor(P[:], P[:], float(CAP), P[:],
                                           op0=Op.is_le, op1=Op.mult)
            pos16 = big.tile([128, SK], I16)
            nc.scalar.activation(pos16[:], P[:], Act.Identity,
                                 bias=bc["bm1"][:], scale=bc["s1"][:])
            vidx = big.tile([128, 2 * SK], I16)
            vv = vidx[:].rearrange("p (j t) -> p j t", t=2)
            nc.scalar.activation(vv[:, :, 0], P[:], Act.Identity,
                                 bias=bc["bm2"][:], scale=bc["s2"][:])
            nc.scalar.activation(vv[:, :, 1], P[:], Act.Identity,
                                 bias=bc["bm1"][:], scale=bc["s2"][:])

            # compaction
            jc = sm.tile([128, CAP], U16)
            nc.gpsimd.local_scatter(jc[:], iota16[:], pos16[:],
                                    channels=128, num_elems=CAP, num_idxs=SK)
            vbuf = sm.tile([128, CAP], F32)
            nc.gpsimd.local_scatter(vbuf[:].bitcast(U16), xt[:].bitcast(U16),
                                    vidx[:], channels=128, num_elems=2 * CAP,
                                    num_idxs=2 * SK)
            padm = sm.tile([128, CAP], U8)
            nc.vector.tensor_tensor(padm[:], slotc_f[:],
                                    cntt[:].to_broadcast([128, CAP]), op=Op.is_gt)
            nc.vector.copy_predicated(vbuf[:], padm[:], negbig[:])

            # extraction
            vals = sm.tile([128, K], F32)
            posb = sm.tile([128, K], U32)
            for i in range(K // 8):
                c8 = slice(i * 8, (i + 1) * 8)
                nc.vector.max(vals[:, c8], vbuf[:])
                nc.vector.max_index(posb[:, c8], vals[:, c8], vbuf[:])
                if i < K // 8 - 1:
                    nc.vector.match_replace(out=vbuf[:], in_to_replace=vals[:, c8],
                                            in_values=vbuf[:], imm_value=SENTINEL)

            # rank -> original column index
            posb16 = sm.tile([128, K], I16)
            nc.scalar.copy(posb16[:], posb[:])
            inv = sm.tile([128, CAP], U16)
            nc.gpsimd.local_scatter(inv[:], slot1_u16[:], posb16[:],
                                    channels=128, num_elems=CAP, num_idxs=K)
            idxs2 = sm.tile([128, CAP], I16)
            nc.scalar.activation(idxs2[:], inv[:], Act.Identity,
                                 bias=bc["bm1"][:], scale=bc["s1"][:])
            outj = sm.tile([128, K], U16)
            nc.gpsimd.local_scatter(outj[:], jc[:], idxs2[:],
                                    channels=128, num_elems=K, num_idxs=CAP)
            tailm = sm.tile([128, K], U8)
            nc.vector.tensor_tensor(tailm[:], slotc_f[:, :K],
                                    cntt[:].to_broadcast([128, K]), op=Op.is_gt)
            nc.vector.copy_predicated(outj[:], tailm[:], slotk_u16[:])
            idxt = opool.tile([128, K], I32)
            nc.scalar.copy(idxt[:], outj[:])
            nc.sync.dma_start(idx_d.ap()[rows, :], idxt[:])

            # mask
            mask16 = big.tile([128, n_chunks * 1024], U16)
            cidx = sm.tile([128, K], I16)
            g = sm.tile([128, K], F32)
            for m in range(n_chunks):
                nc.vector.tensor_scalar(g[:], outj[:], float(1024 * (m + 1)), 4096.0,
                                        op0=Op.is_ge, op1=Op.mult)
                nc.vector.scalar_tensor_tensor(cidx[:], outj[:], float(-1024 * m),
                                               g[:], op0=Op.add, op1=Op.subtract)
                nc.gpsimd.local_scatter(mask16[:, m * 1024 : (m + 1) * 1024],
                                        ones16[:], cidx[:],
                                        channels=128, num_elems=1024, num_idxs=K)
            maskt = opool.tile([128, SK], U8)
            nc.scalar.copy(maskt[:], mask16[:, :SK])
            nc.sync.dma_start(mask_d.ap()[rows, :], maskt[:])

    return nc


def _get_compiled():
    if "nc" not in _CACHE:
        from concourse import bacc

        nc = bacc.Bacc("TRN2", target_bir_lowering=False, debug=False)
        _build(nc)
        nc.compile()
        _CACHE["nc"] = nc
    return _CACHE["nc"]


# ----------------------------------------------------------------------
# exact host fallback for rows where the prior failed (never expected)
# ----------------------------------------------------------------------
def _host_row(xrow, q, k):
    scores = xrow.copy()
    scores[q + 1 :] = np.float32(-1e9)
    idx = np.argsort(-scores, kind="stable")[:k].astype(np.int32)
    mask = np.zeros(xrow.shape[0], dtype=np.uint8)
    mask[idx] = 1
    return mask, idx


def _host_full(index_scores, top_k):
    b, sq, sk = index_scores.shape
    causal = np.triu(np.ones((sq, sk), dtype=bool), k=1)
    scores = np.where(causal[None], np.float32(-1e9), index_scores)
    kk = min(int(top_k), sk)
    idx = np.argsort(-scores, axis=-1, kind="stable")[:, :, :kk].astype(np.int32)
    mask = np.zeros((b, sq, sk), dtype=bool)
    bb = np.arange(b)[:, None, None]
    qq = np.arange(sq)[None, :, None]
    mask[bb, qq, idx] = True
    sparsity = np.float32(1.0) - np.float32(mask.sum()) / np.float32(mask.size)
    return mask, idx, sparsity


# ----------------------------------------------------------------------
# entry point
# ----------------------------------------------------------------------
def kernel(index_scores, top_k):
    index_scores = np.asarray(index_scores, dtype=np.float32)
    if index_scores.shape != (B, SQ, SK) or int(top_k) != K:
        return _host_full(index_scores, int(top_k))

    from concourse.bass_utils import run_bass_kernel_spmd

    nc = _get_compiled()
    T = _threshold_prior(SQ, K)
    in_maps = []
    for c in range(NCORES):
        xs = np.ascontiguousarray(
            index_scores[:, c * QCHUNK : (c + 1) * QCHUNK, :]
        ).reshape(R, SK)
        q = np.tile(np.arange(c * QCHUNK, (c + 1) * QCHUNK, dtype=np.int32), B)
        cm = (np.arange(SK, dtype=np.int32)[None, :] <= q[:, None]).astype(np.uint8)
        tvs = np.tile(T[c * QCHUNK : (c + 1) * QCHUNK], B)
        in_maps.append({
            "x": xs,
            "cm": cm,
            "tv": tvs[:, None].astype(np.float32),
        })

    res = run_bass_kernel_spmd(nc, in_maps, core_ids=list(range(NCORES)))

    mask = np.empty((B, SQ, SK), dtype=np.uint8)
    idx = np.empty((B, SQ, K), dtype=np.int32)
    cnt = np.empty((B, SQ), dtype=np.float32)
    for c, r in enumerate(res.results):
        qs = slice(c * QCHUNK, (c + 1) * QCHUNK)
        mask[:, qs, :] = r["mask"].reshape(B, QCHUNK, SK)
        idx[:, qs, :] = r["idx"].reshape(B, QCHUNK, K)
        cnt[:, qs] = r["cnt"].reshape(B, QCHUNK)

    qarr = np.arange(SQ, dtype=np.float32)[None, :]
    valid = ((cnt >= K) | (cnt == qarr + 1)) & (cnt <= CAP)
    if not valid.all():
        for bb, qq in np.argwhere(~valid):
            mrow, irow = _host_row(index_scores[bb, qq], int(qq), K)
            mask[bb, qq] = mrow
            idx[bb, qq] = irow

    maskb = mask.astype(bool)
    sparsity = np.float32(1.0) - np.float32(maskb.sum()) / np.float32(maskb.size)
    return maskb, idx, sparsity
